# revision 25
# baseline (speedup 1.0000x reference)
"""Trainium2 Bass kernel for nn_CustomLoss_87522843558003 (YOLO-style CIoU+BCE loss).

Strategy (data-parallel over batch, 8 cores):
 - Each core processes 8 consecutive batches. Its 8*8400 positions map onto
   128 SBUF partitions as [batch(8) x section(16)] rows of 525 positions,
   processed in 3 chunks of L=175 positions.
 - Channel-interleaved SBUF layout (contiguous DMA); strided multi-dim APs
   do per-channel compute.
 - Anchor argmax via IoU cross-compare with fast reciprocal; selection with
   copy_predicated (first-max semantics); CIoU/BCE computed post-selection.
 - BCE uses exact-0/1 targets as a predication mask: bce = t ? ln(p) : ln(1-p).
 - Per-partition masked sums via accum_out -> tiny [128,12] output per core;
   final normalization on host.
Engine split: GPSIMD takes TT add/sub/mult bulk ops (no STT/max/min/cmp
support in Pool codegen); ACT takes relu/square/arctan/ln; DVE the rest.
"""

import numpy as np

B, A, N, CH = 64, 3, 8400, 15
NCORES = 8
BPC = B // NCORES      # batches per core
SEC = 16               # partition sections per batch
PPART = BPC * SEC      # 128 partitions
POSROW = N // SEC      # 525 positions per partition row
NCHUNK = 3
L = POSROW // NCHUNK   # 175 positions per chunk per row
C11 = CH - 4           # obj + cls channels
EPS = 1e-7

_CACHE = {}


def _build_bass(loop_r=None, level=4, dma_split=2, io_bufs=1, dma_engines=3,
                big_bufs=1, small_bufs=1, post_dve=False, long_bufs=1):
    """loop_r: if set, wrap the body in a device-side For_i loop repeating it
    loop_r times (identical work each pass; used for exec-time measurement).
    level: staged build for perf attribution. 0=DMA+cnt only, 1=+argmax path,
    2=+selection, 3=+ciou, 4=full (bce).
    dma_split: split each input DMA into this many along the free dim (more
    concurrent DMA queues). io_bufs: bufs for the P/T input pools.
    dma_engines: 1=sync only; 2=alternate sync/scalar (two HWDGE rings);
    3=also gpsimd (SWDGE)."""
    import contextlib
    import concourse.tile as tile
    import concourse.mybir as mybir
    from concourse import bacc

    Alu = mybir.AluOpType
    Act = mybir.ActivationFunctionType
    f32 = mybir.dt.float32

    nc = bacc.Bacc("TRN2", target_bir_lowering=False, debug=False,
                   num_devices=NCORES)
    predL = nc.dram_tensor("predL", [BPC, A, N, CH], f32, kind="ExternalInput").ap()
    targL = nc.dram_tensor("targL", [BPC, N, CH], f32, kind="ExternalInput").ap()
    accO = nc.dram_tensor("acc_out", [PPART, 12], f32, kind="ExternalOutput").ap()

    pre = predL.rearrange("b a (s k j) c -> b a s k (j c)", s=SEC, k=NCHUNK, j=L)
    tre = targL.rearrange("b (s k j) c -> b s k (j c)", s=SEC, k=NCHUNK, j=L)

    with tile.TileContext(nc) as tc:
        with (
            tc.tile_pool(name="pP", bufs=io_bufs) as pP,
            tc.tile_pool(name="pT", bufs=io_bufs) as pT,
            tc.tile_pool(name="pS", bufs=big_bufs) as pS,
            tc.tile_pool(name="pM", bufs=small_bufs) as pM,
            tc.tile_pool(name="pL", bufs=long_bufs) as pL,
            tc.tile_pool(name="pAcc", bufs=1) as pAcc,
        ):
            ACC = pAcc.tile([PPART, 12], f32)

            import concourse.mybir as _mb
            if level == -1:
                # jumbo DMA probe: whole-core loads, one DMA each, no compute
                PJ = pS.tile([PPART, BPC * A * N * CH // PPART], f32)
                TJ = pS.tile([PPART, BPC * N * CH // PPART], f32)
                CN = pM.tile([PPART, L], f32)
                loop_cmj = tc.For_i(0, loop_r, 1) if loop_r else contextlib.nullcontext()
                with loop_cmj:
                    nc.sync.dma_start(PJ[:], predL.rearrange("b a n c -> (b a n c)")
                                      .rearrange("(p f) -> p f", p=PPART))
                    nc.scalar.dma_start(TJ[:], targL.rearrange("b n c -> (b n c)")
                                        .rearrange("(p f) -> p f", p=PPART))
                    nc.vector.tensor_scalar(CN[:], TJ[:, 0:L], 1.0, 0.0, Alu.mult,
                                            Alu.add, accum_out=ACC[:, 0:1])
                nc.sync.dma_start(accO, ACC[:])

            loop_cm = (contextlib.nullcontext() if level == -1 else tc.For_i(0, loop_r, 1, staggered_reset=bool(int(__import__('os').environ.get('LOOP_STAG','0'))),
                                hint_engines=(tuple(_mb.EngineType) if int(__import__('os').environ.get('LOOP_HINT','0')) else ()))
                       if (loop_r and level != -1) else contextlib.nullcontext())
            with loop_cm:
              FR = POSROW * CH  # 7875: full row per anchor per partition
              PF = pP.tile([PPART, A * FR], f32)
              TF = pT.tile([PPART, FR], f32)
              dma_eng = [nc.sync, nc.scalar, nc.gpsimd][:dma_engines]
              pre_f = predL.rearrange("b a (s j) c -> b a s (j c)", s=SEC)
              tre_f = targL.rearrange("b (s j) c -> b s (j c)", s=SEC)
              di = 0
              for s0 in range(dma_split):
                  f0 = FR * s0 // dma_split
                  f1 = FR * (s0 + 1) // dma_split
                  for a in range(A):
                      dma_eng[di % len(dma_eng)].dma_start(
                          PF[:, a * FR + f0:a * FR + f1], pre_f[:, a, :, f0:f1])
                      di += 1
                  dma_eng[di % len(dma_eng)].dma_start(
                      TF[:, f0:f1], tre_f[:, :, f0:f1])
                  di += 1
              PrF = PF[:].rearrange("p (a j c) -> p a j c", a=A, c=CH)
              TrF = TF[:].rearrange("p (j c) -> p j c", c=CH)
              for k in range(NCHUNK if level != -1 else 0):
                Pr = PrF[:, :, k * L:(k + 1) * L, :]
                Tr = TrF[:, k * L:(k + 1) * L, :]

                # ---- masked accumulation: CNT always runs (keeps ACC/T live)
                mask = Tr[:, :, 4]
                CNTs = pS.tile([PPART, L], f32)
                nc.vector.tensor_scalar(CNTs[:], mask, 1.0, 0.0, Alu.mult,
                                        Alu.add, accum_out=ACC[:, 9 + k:10 + k])
                if level < 1:
                    continue
                # ---- shared target prep ----
                TWHH = pS.tile([PPART, L * 2], f32)
                TLO = pL.tile([PPART, L * 2], f32)
                THI = pL.tile([PPART, L * 2], f32)
                TSUM = pL.tile([PPART, L * 2], f32)
                TA = pM.tile([PPART, L], f32)
                RTH = pM.tile([PPART, L], f32)
                RATIOT = pM.tile([PPART, L], f32)
                ATANT = pM.tile([PPART, L], f32)
                tlo = TLO[:].rearrange("p (j c) -> p j c", c=2)
                thi = THI[:].rearrange("p (j c) -> p j c", c=2)
                twhh = TWHH[:].rearrange("p (j c) -> p j c", c=2)
                nc.gpsimd.tensor_scalar(twhh, Tr[:, :, 2:4], 0.5, None, Alu.mult)
                nc.gpsimd.tensor_tensor(tlo, Tr[:, :, 0:2], twhh, Alu.subtract)
                nc.gpsimd.tensor_tensor(thi, Tr[:, :, 0:2], twhh, Alu.add)
                nc.gpsimd.tensor_tensor(TSUM[:], TLO[:], THI[:], Alu.add)
                nc.gpsimd.tensor_tensor(TA[:], Tr[:, :, 2], Tr[:, :, 3], Alu.mult)
                nc.vector.reciprocal_approx_fast(RTH[:], Tr[:, :, 3])
                nc.gpsimd.tensor_tensor(RATIOT[:], Tr[:, :, 2], RTH[:], Alu.mult)
                nc.scalar.activation(ATANT[:], RATIOT[:], Act.Arctan)

                # ---- per-anchor argmax path (all anchors fused per op) ----
                SS = pS.tile([PPART, A * L * 6], f32)
                SSr = SS[:].rearrange("p (a j c) -> p a j c", a=A, c=6)
                PWHH = pS.tile([PPART, A * L * 2], f32, tag="slotP")
                LT = pS.tile([PPART, A * L * 2], f32, tag="slotL")
                RB = pS.tile([PPART, A * L * 2], f32, tag="slotB")
                WHR = pS.tile([PPART, A * L * 2], f32, tag="slotA")
                WHC = pS.tile([PPART, A * L * 2], f32, tag="slotB")
                pwhh = PWHH[:].rearrange("p (a j c) -> p a j c", a=A, c=2)
                ltr = LT[:].rearrange("p (a j c) -> p a j c", a=A, c=2)
                rbr = RB[:].rearrange("p (a j c) -> p a j c", a=A, c=2)
                whrr = WHR[:].rearrange("p (a j c) -> p a j c", a=A, c=2)
                whcr = WHC[:].rearrange("p (a j c) -> p a j c", a=A, c=2)
                tlob = tlo.unsqueeze(1).broadcast_to([PPART, A, L, 2])
                thib = thi.unsqueeze(1).broadcast_to([PPART, A, L, 2])

                nc.scalar.mul(pwhh, Pr[:, :, :, 2:4], 0.5)
                nc.gpsimd.tensor_tensor(SSr[:, :, :, 0:2], Pr[:, :, :, 0:2],
                                        pwhh, Alu.subtract)
                nc.gpsimd.tensor_tensor(SSr[:, :, :, 2:4], Pr[:, :, :, 0:2],
                                        pwhh, Alu.add)
                nc.vector.tensor_tensor(ltr, SSr[:, :, :, 0:2], tlob, Alu.max)
                nc.vector.tensor_tensor(rbr, SSr[:, :, :, 2:4], thib, Alu.min)
                # WHC = relu(rb - lt); sub on GPSIMD, relu on ACT
                nc.gpsimd.tensor_tensor(whrr, rbr, ltr, Alu.subtract)
                nc.scalar.activation(whcr, whrr, Act.Relu)

                PA = pS.tile([PPART, A * L], f32, tag="slotQ")
                S = pS.tile([PPART, A * L], f32)
                par = PA[:].rearrange("p (a j) -> p a j", a=A)
                sr = S[:].rearrange("p (a j) -> p a j", a=A)
                tab = TA[:].unsqueeze(1).broadcast_to([PPART, A, L])
                nc.gpsimd.tensor_tensor(par, Pr[:, :, :, 2], Pr[:, :, :, 3],
                                        Alu.mult)
                nc.gpsimd.tensor_tensor(sr, par, tab, Alu.add)
                nc.vector.tensor_tensor(SSr[:, :, :, 4], whcr[:, :, :, 0],
                                        whcr[:, :, :, 1], Alu.mult)
                nc.gpsimd.tensor_tensor(SSr[:, :, :, 5], sr, SSr[:, :, :, 4],
                                        Alu.subtract)

                # ---- argmax masks ----
                RU = pS.tile([PPART, A * L], f32)
                Q = pS.tile([PPART, A * L], f32, tag="slotQ")
                rur = RU[:].rearrange("p (a j) -> p a j", a=A)
                qr = Q[:].rearrange("p (a j) -> p a j", a=A)
                nc.vector.reciprocal_approx_fast(rur, SSr[:, :, :, 5])
                nc.vector.tensor_tensor(qr, SSr[:, :, :, 4], rur, Alu.mult)
                G2 = pM.tile([PPART, 2 * L], f32)
                G20 = pM.tile([PPART, L], f32)
                N21 = pM.tile([PPART, L], f32)
                W1 = pM.tile([PPART, L], f32)
                W2 = pM.tile([PPART, L], f32)
                g2r = G2[:].rearrange("p (g j) -> p g j", g=2)
                nc.vector.tensor_tensor(g2r, qr[:, 1:3], qr[:, 0:2], Alu.is_gt)
                nc.vector.tensor_tensor(G20[:], qr[:, 2], qr[:, 0], Alu.is_gt)
                nc.gpsimd.tensor_scalar(N21[:], g2r[:, 1], -1.0, 1.0,
                                        Alu.mult, Alu.add)
                nc.gpsimd.tensor_tensor(W1[:], g2r[:, 0], N21[:], Alu.mult)
                nc.gpsimd.tensor_tensor(W2[:], G20[:], g2r[:, 1], Alu.mult)

                if level < 2:
                    continue
                # ---- selection (anchor0 blocks overwritten in place) ----
                w1i = W1[:].bitcast(mybir.dt.int32)
                w2i = W2[:].bitcast(mybir.dt.int32)
                w1b6 = w1i.unsqueeze(2).broadcast_to([PPART, L, 6])
                w2b6 = w2i.unsqueeze(2).broadcast_to([PPART, L, 6])
                nc.vector.copy_predicated(SSr[:, 0], w1b6, SSr[:, 1])
                nc.vector.copy_predicated(SSr[:, 0], w2b6, SSr[:, 2])
                w1b11 = w1i.unsqueeze(2).broadcast_to([PPART, L, C11])
                w2b11 = w2i.unsqueeze(2).broadcast_to([PPART, L, C11])
                nc.vector.copy_predicated(Pr[:, 0, :, 4:CH], w1b11, Pr[:, 1, :, 4:CH])
                nc.vector.copy_predicated(Pr[:, 0, :, 4:CH], w2b11, Pr[:, 2, :, 4:CH])

                SEL = SSr[:, 0]             # [p, j, 6]: x1 y1 x2 y2 i u
                SELP11 = Pr[:, 0, :, 4:CH]  # [p, j, 11]: obj+cls selected

                if level < 3:
                    continue
                # ---- post-selection ciou ----
                DEN = pM.tile([PPART, L], f32, tag="q1")
                RIOU = pM.tile([PPART, L], f32, tag="q2")
                IOU = pM.tile([PPART, L], f32, tag="q3")
                OMI = pM.tile([PPART, L], f32)
                nc.vector.tensor_scalar(DEN[:], SEL[:, :, 5], EPS, None, Alu.add)
                nc.vector.reciprocal_approx_fast(RIOU[:], DEN[:])
                (nc.vector if post_dve else nc.gpsimd).tensor_tensor(IOU[:], SEL[:, :, 4], RIOU[:], Alu.mult)
                nc.vector.tensor_scalar(OMI[:], IOU[:], -1.0, 1.0, Alu.mult, Alu.add)

                CLO = pS.tile([PPART, L * 2], f32, tag="slotL")
                CHI = pS.tile([PPART, L * 2], f32, tag="slotB")
                CWHD = pS.tile([PPART, L * 4], f32, tag="slotP")
                SQ = pS.tile([PPART, L * 4], f32, tag="slotA")
                SP = pS.tile([PPART, L * 2], f32, tag="slotL")
                SELWH = pS.tile([PPART, L * 2], f32, tag="slotB")
                clor = CLO[:].rearrange("p (j c) -> p j c", c=2)
                chir = CHI[:].rearrange("p (j c) -> p j c", c=2)
                cwhdr = CWHD[:].rearrange("p (g j c) -> p g j c", g=2, c=2)
                sqr = SQ[:].rearrange("p (g j c) -> p g j c", g=2, c=2)
                spr = SP[:].rearrange("p (j c) -> p j c", c=2)
                selwhr = SELWH[:].rearrange("p (j c) -> p j c", c=2)
                nc.vector.tensor_tensor(clor, SEL[:, :, 0:2], tlo, Alu.min)
                nc.vector.tensor_tensor(chir, SEL[:, :, 2:4], thi, Alu.max)
                nc.gpsimd.tensor_tensor(cwhdr[:, 0], chir, clor, Alu.subtract)
                nc.gpsimd.tensor_tensor(spr, SEL[:, :, 0:2], SEL[:, :, 2:4],
                                        Alu.add)
                nc.gpsimd.tensor_tensor(
                    cwhdr[:, 1], spr, TSUM[:].rearrange("p (j c) -> p j c", c=2),
                    Alu.subtract)
                nc.scalar.square(SQ[:], CWHD[:])
                nc.gpsimd.tensor_tensor(selwhr, SEL[:, :, 2:4], SEL[:, :, 0:2],
                                        Alu.subtract)

                DIAG = pM.tile([PPART, L], f32, tag="q4")
                RDIAG = pM.tile([PPART, L], f32, tag="q5")
                CDR = pM.tile([PPART, L], f32, tag="q6")
                QD = pM.tile([PPART, L], f32, tag="q7")
                DIOU = pM.tile([PPART, L], f32)
                nc.vector.scalar_tensor_tensor(DIAG[:], sqr[:, 0, :, 0], EPS,
                                               sqr[:, 0, :, 1], Alu.add, Alu.add)
                nc.vector.reciprocal_approx_fast(RDIAG[:], DIAG[:])
                (nc.vector if post_dve else nc.gpsimd).tensor_tensor(CDR[:], sqr[:, 1, :, 0], sqr[:, 1, :, 1], Alu.add)
                (nc.vector if post_dve else nc.gpsimd).tensor_tensor(QD[:], CDR[:], RDIAG[:], Alu.mult)
                nc.vector.scalar_tensor_tensor(DIOU[:], QD[:], 0.25, OMI[:],
                                               Alu.mult, Alu.add)

                RH = pM.tile([PPART, L], f32, tag="q4")
                RATIO = pM.tile([PPART, L], f32, tag="q8")
                ATANP = pM.tile([PPART, L], f32, tag="q9")
                DV = pM.tile([PPART, L], f32, tag="q5")
                V = pM.tile([PPART, L], f32)
                nc.vector.reciprocal_approx_fast(RH[:], selwhr[:, :, 1])
                (nc.vector if post_dve else nc.gpsimd).tensor_tensor(RATIO[:], selwhr[:, :, 0], RH[:], Alu.mult)
                nc.scalar.activation(ATANP[:], RATIO[:], Act.Arctan)
                (nc.vector if post_dve else nc.gpsimd).tensor_tensor(DV[:], ATANT[:], ATANP[:], Alu.subtract)
                nc.scalar.activation(V[:], DV[:], Act.Square,
                                     scale=float(2.0 / np.pi))

                ADEN = pM.tile([PPART, L], f32, tag="q1")
                RADEN = pM.tile([PPART, L], f32, tag="q2")
                ALPHA = pM.tile([PPART, L], f32, tag="q3")
                AV = pM.tile([PPART, L], f32, tag="q6")
                CIOUP = pM.tile([PPART, L], f32, tag="q7")
                nc.vector.scalar_tensor_tensor(ADEN[:], V[:], EPS, OMI[:],
                                               Alu.add, Alu.add)
                nc.vector.reciprocal_approx_fast(RADEN[:], ADEN[:])
                (nc.vector if post_dve else nc.gpsimd).tensor_tensor(ALPHA[:], V[:], RADEN[:], Alu.mult)
                (nc.vector if post_dve else nc.gpsimd).tensor_tensor(AV[:], ALPHA[:], V[:], Alu.mult)
                (nc.vector if post_dve else nc.gpsimd).tensor_tensor(CIOUP[:], DIOU[:], AV[:], Alu.add)

                B3s = pS.tile([PPART, L], f32, tag="slotB")
                nc.vector.scalar_tensor_tensor(B3s[:], CIOUP[:], 1.0, mask,
                                               Alu.mult, Alu.mult,
                                               accum_out=ACC[:, 6 + k:7 + k])
                if level < 4:
                    continue
                # ---- bce on selected obj+cls ----
                bf16 = mybir.dt.bfloat16
                LOGP = pS.tile([PPART, L * C11], bf16)
                LM = pL.tile([PPART, L * C11], bf16)
                logpr = LOGP[:].rearrange("p (j c) -> p j c", c=C11)
                lmr = LM[:].rearrange("p (j c) -> p j c", c=C11)
                nc.scalar.activation(logpr, SELP11, Act.Ln)
                nc.scalar.activation(lmr, SELP11, Act.Ln, bias=1.0, scale=-1.0)
                t11i = TF[:].bitcast(mybir.dt.int32).rearrange(
                    "p (j c) -> p j c", c=CH)[:, k * L:(k + 1) * L, 4:CH]
                nc.vector.copy_predicated(lmr, t11i, logpr)

                # ---- masked accumulation (accum_out -> ACC columns) ----
                maskb = Tr[:, :, 4:5].broadcast_to([PPART, L, C11])
                B2s = pS.tile([PPART, L], f32, tag="slotL")
                nc.vector.scalar_tensor_tensor(logpr, lmr, -0.1, maskb,
                                               Alu.mult, Alu.mult,
                                               accum_out=ACC[:, k:k + 1])
                nc.vector.scalar_tensor_tensor(B2s[:], lmr[:, :, 0], -0.9, mask,
                                               Alu.mult, Alu.mult,
                                               accum_out=ACC[:, 3 + k:4 + k])

            nc.sync.dma_start(accO, ACC[:])

    nc.compile()
    return nc


def kernel(pred, target):
    pred = np.ascontiguousarray(np.asarray(pred, dtype=np.float32))
    target = np.ascontiguousarray(np.asarray(target, dtype=np.float32))
    assert pred.shape == (B, A, N, CH) and target.shape == (B, N, CH)

    if "nc" not in _CACHE:
        _CACHE["nc"] = _build_bass()
    nc = _CACHE["nc"]

    from concourse import bass_utils

    in_maps = []
    for c in range(NCORES):
        lo, hi = c * BPC, (c + 1) * BPC
        in_maps.append({
            "predL": np.ascontiguousarray(pred[lo:hi]),
            "targL": np.ascontiguousarray(target[lo:hi]),
        })

    res = bass_utils.run_bass_kernel_spmd(nc, in_maps, core_ids=list(range(NCORES)))
    _CACHE["last_results"] = res

    per_batch_num = []
    per_batch_cnt = []
    for c in range(NCORES):
        acc = res.results[c]["acc_out"].astype(np.float32)   # [128, 12]
        s_part = acc[:, 0:9].sum(axis=1, dtype=np.float32)   # [128]
        c_part = acc[:, 9:12].sum(axis=1, dtype=np.float32)
        per_batch_num.append(s_part.reshape(BPC, SEC).sum(axis=1, dtype=np.float32))
        per_batch_cnt.append(c_part.reshape(BPC, SEC).sum(axis=1, dtype=np.float32))
    S_b = np.concatenate(per_batch_num).astype(np.float32)   # [64]
    C_b = np.concatenate(per_batch_cnt).astype(np.float32)
    loss = np.mean((S_b / C_b).astype(np.float32), dtype=np.float32)
    return np.float32(loss)
